# revision 34
# baseline (speedup 1.0000x reference)
"""Distributed TRN2 Bass kernel for NSA-style sparse attention.

Problem: b=1, s=2048, d=2048, 16 heads x 128 dim, f32.
  q/k/v = x @ w{q,k,v}.T ; interleaved RoPE on q,k ;
  compressed KV = mean-pool of 16 post-RoPE tokens ;
  joint softmax over [causal compressed blocks ; 256-token sliding window] ;
  out = (p @ [cv;v]) @ wo.T

Sharding: 2 heads per core (column-parallel wq/wk/wv), x replicated.
Output projection: each core ends up owning two interleaved 128-row pieces
of the final output (rows 128c..128c+128 of each s-half).  The attention
outputs are produced directly in [dims, s] orientation (PV matmul with v as
lhsT; softmax 1/l folded into the p-transpose via a diag(1/l) rhs), so the
AllToAll exchanges lhsT-ready tiles and the post-collective DMAs are plain
copies.  Two half-size AllToAlls: the first fires mid-loop and its output
projection overlaps the remaining attention chunks.

Precision: matmul operands in bf16 (f32 PSUM accumulation), softmax stats
and masks in f32.
"""
import sys, os, types

sys.path.insert(0, "/opt/trn_rl_repo")
import numpy as np

S = 2048        # sequence length
D = 2048        # model dim
H = 16          # heads
DH = 128        # head dim
RATIO = 16      # compress ratio
WINDOW = 256    # sliding window
NBLK = S // RATIO          # 128 compressed blocks
ROPE_BASE = 10000.0
NCORES = 8
HPC = H // NCORES          # 2 heads per core
CHUNK = 512                # s-columns per pipeline step
NCHUNK = S // CHUNK        # 4
KT = D // 128              # 16 contraction tiles
NEG = -1e30


def _setup_ntff_hook():
    try:
        import antenv
        if "antenv.axon_hooks" not in sys.modules:
            m = types.ModuleType("antenv.axon_hooks")
            m._hook = None
            m.set_axon_ntff_profile_hook = lambda h: setattr(m, "_hook", h)
            m.get_axon_ntff_profile_hook = lambda: m._hook
            sys.modules["antenv.axon_hooks"] = m
            antenv.axon_hooks = m
        if "/root/.axon_site" not in sys.path:
            sys.path.insert(0, "/root/.axon_site")
        from trn_agent_boot.trn_boot import _ntff_profile_via_ctypes
        hook = _ntff_profile_via_ctypes("/opt/axon/libaxon_pjrt.so")
        sys.modules["antenv.axon_hooks"].set_axon_ntff_profile_hook(hook)
    except Exception:
        pass


def build():
    import concourse.bass as bass
    import concourse.mybir as mybir
    from concourse import bacc, tile
    from concourse.masks import make_identity

    F32 = mybir.dt.float32
    BF16 = mybir.dt.bfloat16
    Alu = mybir.AluOpType
    Act = mybir.ActivationFunctionType
    AX = mybir.AxisListType

    STAGE = int(os.environ.get("KERNEL_STAGE", "5"))
    ATT = int(os.environ.get("KERNEL_ATT", "6"))
    nc = bacc.Bacc(None, target_bir_lowering=False, debug=False)

    # pre-tiled on host: contiguous 2D DMAs into the SBUF layouts
    xt_e = nc.declare_dram_parameter("xt", [NCHUNK, 128, KT * CHUNK], BF16,
                                     isOutput=False)
    wqt_e = nc.declare_dram_parameter("wqt", [128, KT * 256], BF16, isOutput=False)
    wkt_e = nc.declare_dram_parameter("wkt", [128, KT * 256], BF16, isOutput=False)
    wvt_e = nc.declare_dram_parameter("wvt", [128, KT * 256], BF16, isOutput=False)
    wot_e = nc.declare_dram_parameter("wot", [8, 128, 8 * 512], BF16,
                                      isOutput=False)
    cos_e = nc.declare_dram_parameter("cos", [DH, S], BF16, isOutput=False)
    sins_e = nc.declare_dram_parameter("sins", [DH, S], BF16, isOutput=False)
    swap_e = nc.declare_dram_parameter("swapm", [DH, DH], BF16, isOutput=False)
    maskc_e = nc.declare_dram_parameter("maskc", [128, 248], BF16, isOutput=False)
    maskw_e = nc.declare_dram_parameter("maskw", [3, 128, 384], BF16, isOutput=False)
    out_e = nc.declare_dram_parameter("out", [S // NCORES, D], F32, isOutput=True)
    DBG = bool(os.environ.get("KERNEL_DBG"))
    if DBG:
        dbg_e = nc.declare_dram_parameter("dbg", [4, 8 * HPC * DH, 128], BF16,
                                          isOutput=True)
        dbg2_e = nc.declare_dram_parameter("dbg2", [16, HPC, 128, 2], F32,
                                           isOutput=True)
        dbg3_e = nc.declare_dram_parameter("dbg3", [2, 128, KT * 128], BF16,
                                           isOutput=True)

    scale = float(DH) ** -0.5

    with tile.TileContext(nc) as tc:
        with (
            tc.tile_pool(name="const", bufs=1) as constp,
            tc.tile_pool(name="wpool", bufs=1) as wpool,
            tc.tile_pool(name="wopool", bufs=1) as wopool,
            tc.tile_pool(name="xstream", bufs=2) as xstream,
            tc.tile_pool(name="work", bufs=2) as work,
            tc.tile_pool(name="ps", bufs=2, space="PSUM") as ps,
            tc.tile_pool(name="dram", bufs=1, space="DRAM") as dram,
        ):
            # ---------- startup DMAs ----------
            # sync (SP hwdge) queue: x chunk 0 quarters interleaved with
            # wq/wk halves so the first projection matmuls start ~15us in.
            # gpsimd (SWDGE) queue: small consts, then wv, then wo tiles.
            wq_sb = wpool.tile([128, KT * 256], BF16, tag="wq")
            wk_sb = wpool.tile([128, KT * 256], BF16, tag="wk")
            wv_sb = wpool.tile([128, KT * 256], BF16, tag="wv")

            x_tiles = {}
            x_sb0 = xstream.tile([128, KT * CHUNK], BF16, tag="xt", name="x0")
            x_tiles[0] = x_sb0
            XQ = KT * CHUNK // 4
            WH = KT * 256 // 2
            WQ4 = KT * 256 // 4
            for qtr in range(4):
                nc.sync.dma_start(wq_sb[:, WQ4 * qtr:WQ4 * (qtr + 1)],
                                  wqt_e[:, WQ4 * qtr:WQ4 * (qtr + 1)])
                nc.sync.dma_start(x_sb0[:, XQ * qtr:XQ * (qtr + 1)],
                                  xt_e[0][:, XQ * qtr:XQ * (qtr + 1)])
            for qtr in range(4):
                nc.sync.dma_start(wk_sb[:, WQ4 * qtr:WQ4 * (qtr + 1)],
                                  wkt_e[:, WQ4 * qtr:WQ4 * (qtr + 1)])

            cos_sb = constp.tile([DH, S], BF16, tag="cos")
            sins_sb = constp.tile([DH, S], BF16, tag="sins")
            swap_sb = constp.tile([DH, DH], BF16, tag="swap")
            identb = constp.tile([128, 128], BF16, tag="identb")
            maskc_sb = constp.tile([128, 248], BF16, tag="maskc")
            maskw_sb = constp.tile([128, 3 * 384], BF16, tag="maskw")
            nc.gpsimd.dma_start(swap_sb[:], swap_e[:])
            nc.gpsimd.dma_start(maskc_sb[:], maskc_e[:])
            nc.gpsimd.dma_start(cos_sb[:], cos_e[:])
            nc.gpsimd.dma_start(sins_sb[:], sins_e[:])
            nc.gpsimd.dma_start(wv_sb[:], wvt_e[:])
            nc.gpsimd.dma_start(
                maskw_sb[:].rearrange("p (t f) -> p t f", t=3),
                maskw_e[:].rearrange("t p f -> p t f"),
            )
            make_identity(nc, identb[:])

            def maskw_t(t):
                i = min(t, 2)
                return maskw_sb[:, 384 * i:384 * (i + 1)]

            # bp staging reuses the wq/wk weight tiles (dead after the last
            # projection): plain-tensor WAR tracking, avoids the pool
            # allocator's buggy release-dep path for recycled slots
            bp_static = [wq_sb[:, 0:KT * 128], wk_sb[:, 0:KT * 128]]

            wo_tiles = []
            for i in range(8):
                wsb = wopool.tile([128, 8 * 512], BF16, tag=f"wo{i}")
                nc.gpsimd.dma_start(wsb[:], wot_e[i])
                wo_tiles.append(wsb)

            # ---------- persistent per-head state ----------
            # kt_full: [dh, 256 zero-pad + s] rotated keys
            kt_full = [work.tile([DH, WINDOW + S], BF16, tag=f"ktf{h}", bufs=1,
                                 name=f"ktf{h}") for h in range(HPC)]
            # vrow: row-major v, col block w = s-tile (w-2); first 2 blocks zero
            vrow = [work.tile([128, (2 + S // 128) * DH], BF16, tag=f"vrow{h}",
                              bufs=1, name=f"vrow{h}") for h in range(HPC)]
            ckt = [work.tile([DH, NBLK], BF16, tag=f"ckt{h}", bufs=1,
                             name=f"ckt{h}") for h in range(HPC)]
            cvrow = [work.tile([NBLK, DH], BF16, tag=f"cvrow{h}", bufs=1,
                               name=f"cvrow{h}") for h in range(HPC)]
            cvt_acc = [work.tile([DH, NBLK], BF16, tag=f"cvt{h}", bufs=1,
                                 name=f"cvt{h}") for h in range(HPC)]
            for h in range(HPC):
                nc.vector.memset(kt_full[h][:, 0:WINDOW], 0.0)
                nc.vector.memset(vrow[h][:, 0:2 * DH], 0.0)
                nc.vector.memset(ckt[h][:], 0.0)
                nc.vector.memset(cvrow[h][:], 0.0)
                nc.vector.memset(cvt_acc[h][:], 0.0)

            # a2a bounce buffers, [dims, s] orientation, one pair per s-half.
            # rows 256*i .. 256*i+256 of a2a_in[u] = my 2x128 dims for the
            # s-tile owned by core i in half u (s-tile index 8u+i).
            a2a_in = [dram.tile([8 * HPC * DH, 128], BF16, name=f"a2ain{u}")
                      for u in range(2)]
            a2a_out = [dram.tile([8 * HPC * DH, 128], BF16, name=f"a2aout{u}")
                       for u in range(2)]

            def stream_x(c):
                x_sb = xstream.tile([128, KT * CHUNK], BF16, tag="xt")
                nc.sync.dma_start(x_sb[:], xt_e[c])
                x_tiles[c] = x_sb

            # ---------- projection block for chunk c ----------
            # returns (qt, [6 emitter closures]): Q/K/V x 2 heads, used to
            # interleave with the previous chunk's attention stages so the
            # PE queue never starves (keeps the p-state ramp at 2.4GHz)
            def proj_block(c):
                col0 = CHUNK * c
                cols = slice(col0, col0 + CHUNK)
                if c + 1 < NCHUNK:
                    stream_x(c + 1)
                x_sb = x_tiles[c]

                def project(w_sb, h):
                    acc = ps.tile([128, CHUNK], F32, tag="acc")
                    for kk in range(KT):
                        nc.tensor.matmul(
                            acc[:],
                            w_sb[:, 256 * kk + 128 * h:256 * kk + 128 * (h + 1)],
                            x_sb[:, CHUNK * kk:CHUNK * (kk + 1)],
                            start=(kk == 0), stop=(kk == KT - 1),
                        )
                    return acc

                def rope(acc, dest_ap):
                    raw = work.tile([DH, CHUNK], BF16, tag="qraw")
                    nc.scalar.copy(raw[:], acc[:])
                    sw_ps = ps.tile([DH, CHUNK], F32, tag="acc")
                    nc.tensor.matmul(sw_ps[:], swap_sb[:], raw[:],
                                     start=True, stop=True)
                    t1 = work.tile([DH, CHUNK], F32, tag="rope1", bufs=1)
                    nc.vector.tensor_tensor(t1[:], raw[:], cos_sb[:, cols],
                                            Alu.mult)
                    t2 = work.tile([DH, CHUNK], F32, tag="rope2", bufs=1)
                    nc.vector.tensor_tensor(t2[:], sw_ps[:], sins_sb[:, cols],
                                            Alu.mult)
                    nc.vector.tensor_tensor(dest_ap, t1[:], t2[:], Alu.add)

                qt = [work.tile([DH, CHUNK], BF16, tag="qt", bufs=4,
                                name=f"qt{c}_{h}") for h in range(HPC)]

                def g_q(h):
                    rope(project(wq_sb, h), qt[h][:])

                def g_k(h):
                    kdst = kt_full[h][:, WINDOW + col0:WINDOW + col0 + CHUNK]
                    rope(project(wk_sb, h), kdst)
                    cks = work.tile([DH, CHUNK // RATIO], F32, tag="cks")
                    nc.vector.tensor_reduce(
                        cks[:], kdst.rearrange("p (b r) -> p b r", r=RATIO),
                        AX.X, Alu.add)
                    nc.vector.tensor_scalar_mul(
                        ckt[h][:, col0 // RATIO:(col0 + CHUNK) // RATIO],
                        cks[:], 1.0 / RATIO)

                def g_v(h):
                    acc_v = project(wv_sb, h)
                    vt = work.tile([DH, CHUNK], BF16, tag="vt")
                    nc.scalar.copy(vt[:], acc_v[:])
                    cvs = work.tile([DH, CHUNK // RATIO], F32, tag="cks")
                    nc.vector.tensor_reduce(
                        cvs[:], vt[:].rearrange("p (b r) -> p b r", r=RATIO),
                        AX.X, Alu.add)
                    nc.vector.tensor_scalar_mul(
                        cvt_acc[h][:, col0 // RATIO:(col0 + CHUNK) // RATIO],
                        cvs[:], 1.0 / RATIO)
                    cv_ps = ps.tile([NBLK, DH], BF16, tag="pt", bufs=1)
                    nc.tensor.transpose(cv_ps[:], cvt_acc[h][:], identb[:])
                    nc.vector.tensor_copy(cvrow[h][:], cv_ps[:])
                    vtr_ps = ps.tile([128, CHUNK], BF16, tag="pt", bufs=1)
                    for tt in range(CHUNK // 128):
                        nc.tensor.transpose(vtr_ps[:, 128 * tt:128 * (tt + 1)],
                                            vt[:, 128 * tt:128 * (tt + 1)], identb[:])
                    st0 = CHUNK // 128 * c  # first raw s-tile of this chunk
                    nc.vector.tensor_copy(
                        vrow[h][:, (st0 + 2) * DH:(st0 + 2) * DH + CHUNK], vtr_ps[:])

                groups = [lambda h=h: g_q(h) for h in range(HPC)]
                groups += [lambda h=h: g_k(h) for h in range(HPC)]
                groups += [lambda h=h: g_v(h) for h in range(HPC)]
                return qt, groups

            # ---------- attention block for chunk c (software-pipelined) ----
            # stage A(t): score matmuls + mask-add + exp/accum + recip + diag
            # stage B(t): p-transpose (rhs = diag(1/l)) + pts copy
            # stage C(t): PV matmuls ([dh, q] orientation) + osb copy + DMA
            def att_block(c, qt):
                tiles = []  # (h, tg, qslice)
                for sub in range(CHUNK // 256):
                    sblk = (CHUNK // 256) * c + sub
                    for h in range(HPC):
                        for tt in range(2):
                            tg = 2 * sblk + tt
                            qs = qt[h][:, 256 * sub + 128 * tt:
                                       256 * sub + 128 * (tt + 1)]
                            tiles.append((h, tg, qs))

                state = {}
                fences = {}
                ot_bank = ps.tile([DH, 512], F32, tag="ot", bufs=1,
                                  name=f"otbank{c}")

                def stage_a(i):
                    if ATT < 1:
                        return
                    h, tg, qs = tiles[i]
                    s_ps = ps.tile([128, 512], F32, tag="sc", bufs=2)
                    nc.tensor.matmul(s_ps[:, 0:128], qs, ckt[h][:],
                                     start=True, stop=True)
                    nc.tensor.matmul(s_ps[:, 128:512], qs,
                                     kt_full[h][:, 128 * tg:128 * tg + 384],
                                     start=True, stop=True)
                    if ATT < 2:
                        return
                    # s_sb = scores*scale + mask; logits are bounded (~|8|)
                    # so exp needs no max subtraction.
                    s_sb = work.tile([128, 512], F32, tag="ssb", bufs=2)
                    nc.vector.scalar_tensor_tensor(
                        s_sb[:, 0:128], s_ps[:, 0:128], scale,
                        maskc_sb[:, 120 - 8 * tg:248 - 8 * tg],
                        Alu.mult, Alu.add)
                    nc.vector.scalar_tensor_tensor(
                        s_sb[:, 128:512], s_ps[:, 128:512], scale,
                        maskw_t(tg), Alu.mult, Alu.add)
                    if ATT < 3:
                        return
                    p_sb = work.tile([128, 512], BF16, tag="psb", bufs=2)
                    lsum = work.tile([128, 1], F32, tag="stat", bufs=16)
                    fences["act"] = nc.scalar.activation(
                        p_sb[:], s_sb[:], Act.Exp, bias=0.0, scale=1.0,
                        accum_out=lsum[:])
                    rl = work.tile([128, 1], F32, tag="stat", bufs=16)
                    nc.vector.reciprocal(rl[:], lsum[:])
                    if DBG:
                        nc.sync.dma_start(dbg2_e[tg, h, :, 0:1], lsum[:])
                        nc.sync.dma_start(dbg2_e[tg, h, :, 1:2], rl[:])
                    if ATT < 4:
                        return
                    diag = work.tile([128, 128], BF16, tag="diag", bufs=2)
                    nc.vector.tensor_scalar_mul(diag[:], identb[:], rl[:])
                    state[i] = (p_sb, diag)

                def stage_b(i):
                    if ATT < 5 or i not in state:
                        return
                    p_sb, diag = state[i]
                    if os.environ.get("KERNEL_NODIAG"):
                        diag = identb
                    # pt = p^T @ diag(1/l): transposed AND normalized
                    pt_ps = ps.tile([128, 512], F32, tag="ptb")
                    for j in range(4):
                        nc.tensor.matmul(pt_ps[:, 128 * j:128 * (j + 1)],
                                         p_sb[:, 128 * j:128 * (j + 1)],
                                         diag[:], start=True, stop=True)
                    pts = work.tile([128, 512], BF16, tag="pts", bufs=2)
                    nc.vector.tensor_copy(pts[:], pt_ps[:])
                    state[i] = pts

                def stage_c(i):
                    if ATT < 6 or i not in state:
                        return
                    h, tg, _ = tiles[i]
                    pts = state.pop(i)
                    # PV in [dh, q] orientation: lhsT = v rows ([kv, dh])
                    o_ps = ot_bank[:, 128 * (i % 4):128 * (i % 4 + 1)]
                    nc.tensor.matmul(o_ps, cvrow[h][:], pts[:, 0:128],
                                     start=True, stop=False)
                    for j in range(3):
                        w = tg - 2 + j  # raw s-tile; vrow block w+2
                        nc.tensor.matmul(
                            o_ps, vrow[h][:, (w + 2) * DH:(w + 3) * DH],
                            pts[:, 128 * (j + 1):128 * (j + 2)],
                            start=False, stop=(j == 2))
                    osb = work.tile([DH, 128], BF16, tag="osb", bufs=2,
                                    name="osb")
                    fences["dve"] = nc.vector.tensor_copy(osb[:], o_ps)
                    u, i8 = tg // 8, tg % 8
                    # spread the 16 per-half writes over both DMA queues so
                    # the collective trigger isn't gated on one serial queue
                    eng = nc.sync if h == 0 else nc.gpsimd
                    fences["dma"] = eng.dma_start(
                        a2a_in[u][256 * i8 + 128 * h:256 * i8 + 128 * (h + 1), :],
                        osb[:])

                n = len(tiles)

                def make_step(i):
                    def step():
                        if i < n:
                            stage_a(i)
                        if 1 <= i <= n:
                            stage_b(i - 1)
                        if 2 <= i:
                            stage_c(i - 2)
                    return step

                return [make_step(i) for i in range(n + 2)], fences

            def trigger_a2a(u):
                if STAGE < 4:
                    return
                nc.gpsimd.collective_compute(
                    "AllToAll", mybir.AluOpType.bypass,
                    replica_groups=[list(range(NCORES))],
                    ins=[a2a_in[u][:].opt()], outs=[a2a_out[u][:].opt()],
                )

            # ---------- output projection for half u ----------
            def bp_dma(u):
                if STAGE < 3 or STAGE == 4:
                    return
                src_t = a2a_out[u] if STAGE >= 4 else a2a_in[u]
                nc.sync.dma_start(
                    bp_static[u].rearrange("p (k s) -> p k s", k=KT),
                    src_t[:].rearrange("(k p) s -> p k s", p=128))

            def out_proj_groups(u):
                if STAGE < 3 or STAGE == 4:
                    def zero(n):
                        z = work.tile([128, 512], F32, tag="outsb")
                        nc.vector.memset(z[:], 0.0)
                        nc.gpsimd.dma_start(
                            out_e[128 * u:128 * (u + 1),
                                  512 * n:512 * (n + 1)], z[:])
                    return [lambda n=n: zero(n) for n in range(4)]
                bp_sb = bp_static[u]

                def group(n):
                    acc = ps.tile([128, 512], F32, tag="acc")
                    for kk in range(KT):
                        wsb = wo_tiles[2 * n + kk // 8]
                        nc.tensor.matmul(
                            acc[:],
                            bp_sb[:, 128 * kk:128 * (kk + 1)],
                            wsb[:, 512 * (kk % 8):512 * (kk % 8 + 1)],
                            start=(kk == 0), stop=(kk == KT - 1),
                        )
                    outsb = work.tile([128, 512], F32, tag="outsb")
                    nc.vector.tensor_copy(outsb[:], acc[:])
                    nc.gpsimd.dma_start(
                        out_e[128 * u:128 * (u + 1), 512 * n:512 * (n + 1)],
                        outsb[:])
                return [lambda n=n: group(n) for n in range(4)]

            # ---------- schedule ----------
            # interleave attention steps of chunk c with projection groups of
            # chunk c+1: the proj matmuls fill the PE queue while the
            # attention mask/exp chain completes, keeping the ramp hot.
            def interleave(steps, groups, slots=(1, 2, 3, 4, 5, 6)):
                gi = 0
                for i, step in enumerate(steps):
                    step()
                    if i + 1 in slots and gi < len(groups):
                        groups[gi]()
                        gi += 1
                while gi < len(groups):
                    groups[gi]()
                    gi += 1

            qt0, pg0 = proj_block(0)
            for g in pg0:
                g()
            qt1, pg1 = proj_block(1)
            steps0, f0 = att_block(0, qt0)
            interleave(steps0, pg1)
            qt2, pg2 = proj_block(2)
            steps1, f1 = att_block(1, qt1)
            interleave(steps1, pg2)
            trigger_a2a(0)
            qt3, pg3 = proj_block(3)
            steps2, f2 = att_block(2, qt2)
            interleave(steps2, pg3)
            steps3, f3 = att_block(3, qt3)
            for s in steps3:
                s()
            trigger_a2a(1)
            bp_dma(0)
            for g in out_proj_groups(0):
                g()
            bp_dma(1)
            for g in out_proj_groups(1):
                g()
    return nc


def _host_inputs(x, wq, wk, wv, wo):
    """Build per-core input maps (numpy)."""
    import ml_dtypes
    BF = ml_dtypes.bfloat16
    xT = x.reshape(S, D).T.astype(BF)          # [D, S]
    xtile = np.ascontiguousarray(
        xT.reshape(KT, 128, NCHUNK, CHUNK).transpose(2, 1, 0, 3)
        .reshape(NCHUNK, 128, KT * CHUNK))
    woT = wo.T.astype(BF)                      # [D, D]
    wotile = np.ascontiguousarray(
        woT.reshape(2, 8, 128, 4, 512).transpose(3, 0, 2, 1, 4)
        .reshape(8, 128, 8 * 512))

    def wtile(w, rows):
        wT = w[rows, :].T.astype(BF)           # [D, 256]
        return np.ascontiguousarray(
            wT.reshape(KT, 128, 256).transpose(1, 0, 2).reshape(128, KT * 256))

    inv = 1.0 / (ROPE_BASE ** (np.arange(0, DH, 2, dtype=np.float32) / DH))
    theta = np.outer(np.arange(S, dtype=np.float32), inv)  # [S, 64]
    cos = np.cos(theta).T  # [64, S]
    sin = np.sin(theta).T
    COS = np.empty((DH, S), np.float32)
    SINS = np.empty((DH, S), np.float32)
    COS[0::2] = cos
    COS[1::2] = cos
    SINS[0::2] = -sin
    SINS[1::2] = sin

    SWAP = np.zeros((DH, DH), np.float32)
    for t in range(DH // 2):
        SWAP[2 * t + 1, 2 * t] = 1.0
        SWAP[2 * t, 2 * t + 1] = 1.0

    ii = np.arange(128)[:, None]
    cnt = (ii + 1) // RATIO  # [128,1]
    w = np.arange(248)[None, :] - 120
    maskc = np.where(w < cnt, 0.0, NEG).astype(np.float32)  # [128,248]

    col = np.arange(384)[None, :]
    base_vis = (ii < col) & (col <= ii + WINDOW)
    maskw = np.empty((3, 128, 384), np.float32)
    for idx, t in enumerate([0, 1, 2]):
        vis = base_vis & (col >= WINDOW - 128 * t if t < 2 else True)
        maskw[idx] = np.where(vis, 0.0, NEG)

    in_maps = []
    for cid in range(NCORES):
        rows = slice(256 * cid, 256 * (cid + 1))
        in_maps.append({
            "xt": xtile,
            "wqt": wtile(wq, rows),
            "wkt": wtile(wk, rows),
            "wvt": wtile(wv, rows),
            "wot": wotile,
            "cos": COS.astype(BF),
            "sins": SINS.astype(BF),
            "swapm": SWAP.astype(BF),
            "maskc": maskc.astype(BF),
            "maskw": maskw.astype(BF),
        })
    return in_maps


_CACHE = {}
LAST_EXEC_NS = None


def kernel(x, wq, wk, wv, wo):
    _setup_ntff_hook()
    from concourse.bass_utils import run_bass_kernel_spmd

    if "nc" not in _CACHE:
        ncb = build()
        if not ncb.is_finalized():
            ncb.finalize()
        _CACHE["nc"] = ncb
    ncb = _CACHE["nc"]

    in_maps = _host_inputs(np.asarray(x), np.asarray(wq), np.asarray(wk),
                           np.asarray(wv), np.asarray(wo))
    trace = bool(os.environ.get("KERNEL_TRACE"))
    res = run_bass_kernel_spmd(ncb, in_maps, list(range(NCORES)), trace=trace)
    globals()["LAST_EXEC_NS"] = res.exec_time_ns
    # core c owns rows 128c..128c+128 of each s-half
    out = np.empty((S, D), np.float32)
    for c in range(NCORES):
        r = res.results[c]["out"]
        out[128 * c:128 * (c + 1)] = r[0:128]
        out[1024 + 128 * c:1024 + 128 * (c + 1)] = r[128:256]
    return out.reshape(1, S, D)


if __name__ == "__main__":
    rng = np.random.default_rng(0)
    x = rng.standard_normal((1, S, D), dtype=np.float32)
    wq = rng.standard_normal((D, D), dtype=np.float32) * D ** -0.5
    wk = rng.standard_normal((D, D), dtype=np.float32) * D ** -0.5
    wv = rng.standard_normal((D, D), dtype=np.float32) * D ** -0.5
    wo = rng.standard_normal((D, D), dtype=np.float32) * D ** -0.5
    out = kernel(x=x, wq=wq, wk=wk, wv=wv, wo=wo)
    print("out", out.shape, out.dtype, np.abs(out).mean())


# revision 35
# speedup vs baseline: 1.0602x; 1.0602x over previous
"""Distributed TRN2 Bass kernel for NSA-style sparse attention.

Problem: b=1, s=2048, d=2048, 16 heads x 128 dim, f32.
  q/k/v = x @ w{q,k,v}.T ; interleaved RoPE on q,k ;
  compressed KV = mean-pool of 16 post-RoPE tokens ;
  joint softmax over [causal compressed blocks ; 256-token sliding window] ;
  out = (p @ [cv;v]) @ wo.T

Sharding: 2 heads per core (column-parallel wq/wk/wv), x replicated.
Output projection: each core ends up owning two interleaved 128-row pieces
of the final output (rows 128c..128c+128 of each s-half).  The attention
outputs are produced directly in [dims, s] orientation (PV matmul with v as
lhsT; softmax 1/l folded into the p-transpose via a diag(1/l) rhs), so the
AllToAll exchanges lhsT-ready tiles and the post-collective DMAs are plain
copies.  Two half-size AllToAlls: the first fires mid-loop and its output
projection overlaps the remaining attention chunks.

Precision: matmul operands in bf16 (f32 PSUM accumulation), softmax stats
and masks in f32.
"""
import sys, os, types

sys.path.insert(0, "/opt/trn_rl_repo")
import numpy as np

S = 2048        # sequence length
D = 2048        # model dim
H = 16          # heads
DH = 128        # head dim
RATIO = 16      # compress ratio
WINDOW = 256    # sliding window
NBLK = S // RATIO          # 128 compressed blocks
ROPE_BASE = 10000.0
NCORES = 8
HPC = H // NCORES          # 2 heads per core
CHUNK = 512                # s-columns per pipeline step
NCHUNK = S // CHUNK        # 4
KT = D // 128              # 16 contraction tiles
NEG = -1e30


def _setup_ntff_hook():
    try:
        import antenv
        if "antenv.axon_hooks" not in sys.modules:
            m = types.ModuleType("antenv.axon_hooks")
            m._hook = None
            m.set_axon_ntff_profile_hook = lambda h: setattr(m, "_hook", h)
            m.get_axon_ntff_profile_hook = lambda: m._hook
            sys.modules["antenv.axon_hooks"] = m
            antenv.axon_hooks = m
        if "/root/.axon_site" not in sys.path:
            sys.path.insert(0, "/root/.axon_site")
        from trn_agent_boot.trn_boot import _ntff_profile_via_ctypes
        hook = _ntff_profile_via_ctypes("/opt/axon/libaxon_pjrt.so")
        sys.modules["antenv.axon_hooks"].set_axon_ntff_profile_hook(hook)
    except Exception:
        pass


def build():
    import concourse.bass as bass
    import concourse.mybir as mybir
    from concourse import bacc, tile
    from concourse.masks import make_identity

    F32 = mybir.dt.float32
    BF16 = mybir.dt.bfloat16
    Alu = mybir.AluOpType
    Act = mybir.ActivationFunctionType
    AX = mybir.AxisListType

    STAGE = int(os.environ.get("KERNEL_STAGE", "5"))
    ATT = int(os.environ.get("KERNEL_ATT", "6"))
    nc = bacc.Bacc(None, target_bir_lowering=False, debug=False)

    # pre-tiled on host: contiguous 2D DMAs into the SBUF layouts
    xt_e = nc.declare_dram_parameter("xt", [NCHUNK, 128, KT * CHUNK], BF16,
                                     isOutput=False)
    wqt_e = nc.declare_dram_parameter("wqt", [128, KT * 256], BF16, isOutput=False)
    wkt_e = nc.declare_dram_parameter("wkt", [128, KT * 256], BF16, isOutput=False)
    wvt_e = nc.declare_dram_parameter("wvt", [128, KT * 256], BF16, isOutput=False)
    wot_e = nc.declare_dram_parameter("wot", [8, 128, 8 * 512], BF16,
                                      isOutput=False)
    cos_e = nc.declare_dram_parameter("cos", [DH, S], BF16, isOutput=False)
    sins_e = nc.declare_dram_parameter("sins", [DH, S], BF16, isOutput=False)
    swap_e = nc.declare_dram_parameter("swapm", [DH, DH], BF16, isOutput=False)
    maskc_e = nc.declare_dram_parameter("maskc", [128, 248], BF16, isOutput=False)
    maskw_e = nc.declare_dram_parameter("maskw", [3, 128, 384], BF16, isOutput=False)
    out_e = nc.declare_dram_parameter("out", [S // NCORES, D], F32, isOutput=True)
    DBG = bool(os.environ.get("KERNEL_DBG"))
    if DBG:
        dbg_e = nc.declare_dram_parameter("dbg", [4, 8 * HPC * DH, 128], BF16,
                                          isOutput=True)
        dbg2_e = nc.declare_dram_parameter("dbg2", [16, HPC, 128, 2], F32,
                                           isOutput=True)
        dbg3_e = nc.declare_dram_parameter("dbg3", [2, 128, KT * 128], BF16,
                                           isOutput=True)

    scale = float(DH) ** -0.5

    with tile.TileContext(nc) as tc:
        with (
            tc.tile_pool(name="const", bufs=1) as constp,
            tc.tile_pool(name="wpool", bufs=1) as wpool,
            tc.tile_pool(name="wopool", bufs=1) as wopool,
            tc.tile_pool(name="xstream", bufs=2) as xstream,
            tc.tile_pool(name="work", bufs=2) as work,
            tc.tile_pool(name="ps", bufs=2, space="PSUM") as ps,
            tc.tile_pool(name="dram", bufs=1, space="DRAM") as dram,
        ):
            # ---------- startup DMAs ----------
            # sync (SP hwdge) queue: x chunk 0 quarters interleaved with
            # wq/wk halves so the first projection matmuls start ~15us in.
            # gpsimd (SWDGE) queue: small consts, then wv, then wo tiles.
            wq_sb = wpool.tile([128, KT * 256], BF16, tag="wq")
            wk_sb = wpool.tile([128, KT * 256], BF16, tag="wk")
            wv_sb = wpool.tile([128, KT * 256], BF16, tag="wv")

            x_tiles = {}
            x_sb0 = xstream.tile([128, KT * CHUNK], BF16, tag="xt", name="x0")
            x_tiles[0] = x_sb0
            XQ = KT * CHUNK // 4
            WH = KT * 256 // 2
            WQ4 = KT * 256 // 4
            for qtr in range(4):
                nc.sync.dma_start(wq_sb[:, WQ4 * qtr:WQ4 * (qtr + 1)],
                                  wqt_e[:, WQ4 * qtr:WQ4 * (qtr + 1)])
                nc.sync.dma_start(x_sb0[:, XQ * qtr:XQ * (qtr + 1)],
                                  xt_e[0][:, XQ * qtr:XQ * (qtr + 1)])
            for qtr in range(4):
                nc.sync.dma_start(wk_sb[:, WQ4 * qtr:WQ4 * (qtr + 1)],
                                  wkt_e[:, WQ4 * qtr:WQ4 * (qtr + 1)])

            cos_sb = constp.tile([DH, S], BF16, tag="cos")
            sins_sb = constp.tile([DH, S], BF16, tag="sins")
            swap_sb = constp.tile([DH, DH], BF16, tag="swap")
            identb = constp.tile([128, 128], BF16, tag="identb")
            maskc_sb = constp.tile([128, 248], BF16, tag="maskc")
            maskw_sb = constp.tile([128, 3 * 384], BF16, tag="maskw")
            nc.gpsimd.dma_start(swap_sb[:], swap_e[:])
            nc.gpsimd.dma_start(maskc_sb[:], maskc_e[:])
            nc.gpsimd.dma_start(cos_sb[:], cos_e[:])
            nc.gpsimd.dma_start(sins_sb[:], sins_e[:])
            nc.gpsimd.dma_start(wv_sb[:], wvt_e[:])
            nc.gpsimd.dma_start(
                maskw_sb[:].rearrange("p (t f) -> p t f", t=3),
                maskw_e[:].rearrange("t p f -> p t f"),
            )
            make_identity(nc, identb[:])

            def maskw_t(t):
                i = min(t, 2)
                return maskw_sb[:, 384 * i:384 * (i + 1)]

            # bp staging reuses the wq/wk weight tiles (dead after the last
            # projection): plain-tensor WAR tracking, avoids the pool
            # allocator's buggy release-dep path for recycled slots
            bp_static = [wq_sb[:, 0:KT * 128], wk_sb[:, 0:KT * 128]]

            wo_tiles = []
            for i in range(8):
                wsb = wopool.tile([128, 8 * 512], BF16, tag=f"wo{i}")
                nc.gpsimd.dma_start(wsb[:], wot_e[i])
                wo_tiles.append(wsb)

            # ---------- persistent per-head state ----------
            # kt_full: [dh, 256 zero-pad + s] rotated keys
            kt_full = [work.tile([DH, WINDOW + S], BF16, tag=f"ktf{h}", bufs=1,
                                 name=f"ktf{h}") for h in range(HPC)]
            # vrow: row-major v, col block w = s-tile (w-2); first 2 blocks zero
            vrow = [work.tile([128, (2 + S // 128) * DH], BF16, tag=f"vrow{h}",
                              bufs=1, name=f"vrow{h}") for h in range(HPC)]
            ckt = [work.tile([DH, NBLK], BF16, tag=f"ckt{h}", bufs=1,
                             name=f"ckt{h}") for h in range(HPC)]
            cvrow = [work.tile([NBLK, DH], BF16, tag=f"cvrow{h}", bufs=1,
                               name=f"cvrow{h}") for h in range(HPC)]
            cvt_acc = [work.tile([DH, NBLK], BF16, tag=f"cvt{h}", bufs=1,
                                 name=f"cvt{h}") for h in range(HPC)]
            for h in range(HPC):
                nc.vector.memset(kt_full[h][:, 0:WINDOW], 0.0)
                nc.vector.memset(vrow[h][:, 0:2 * DH], 0.0)
                nc.vector.memset(ckt[h][:], 0.0)
                nc.vector.memset(cvrow[h][:], 0.0)
                nc.vector.memset(cvt_acc[h][:], 0.0)

            # a2a bounce buffers, [dims, s] orientation, one pair per s-half.
            # rows 256*i .. 256*i+256 of a2a_in[u] = my 2x128 dims for the
            # s-tile owned by core i in half u (s-tile index 8u+i).
            a2a_in = [dram.tile([8 * HPC * DH, 128], BF16, name=f"a2ain{u}")
                      for u in range(2)]
            a2a_out = [dram.tile([8 * HPC * DH, 128], BF16, name=f"a2aout{u}")
                       for u in range(2)]

            def stream_x(c):
                x_sb = xstream.tile([128, KT * CHUNK], BF16, tag="xt")
                nc.sync.dma_start(x_sb[:], xt_e[c])
                x_tiles[c] = x_sb

            # ---------- projection block for chunk c ----------
            # returns (qt, [6 emitter closures]): Q/K/V x 2 heads, used to
            # interleave with the previous chunk's attention stages so the
            # PE queue never starves (keeps the p-state ramp at 2.4GHz)
            def proj_block(c):
                col0 = CHUNK * c
                cols = slice(col0, col0 + CHUNK)
                if c + 1 < NCHUNK:
                    stream_x(c + 1)
                x_sb = x_tiles[c]

                def project(w_sb, h):
                    acc = ps.tile([128, CHUNK], F32, tag="acc")
                    for kk in range(KT):
                        nc.tensor.matmul(
                            acc[:],
                            w_sb[:, 256 * kk + 128 * h:256 * kk + 128 * (h + 1)],
                            x_sb[:, CHUNK * kk:CHUNK * (kk + 1)],
                            start=(kk == 0), stop=(kk == KT - 1),
                        )
                    return acc

                def rope(acc, dest_ap):
                    raw = work.tile([DH, CHUNK], BF16, tag="qraw")
                    nc.scalar.copy(raw[:], acc[:])
                    sw_ps = ps.tile([DH, CHUNK], F32, tag="acc")
                    nc.tensor.matmul(sw_ps[:], swap_sb[:], raw[:],
                                     start=True, stop=True)
                    t1 = work.tile([DH, CHUNK], F32, tag="rope1", bufs=1)
                    nc.vector.tensor_tensor(t1[:], raw[:], cos_sb[:, cols],
                                            Alu.mult)
                    t2 = work.tile([DH, CHUNK], F32, tag="rope2", bufs=1)
                    nc.vector.tensor_tensor(t2[:], sw_ps[:], sins_sb[:, cols],
                                            Alu.mult)
                    nc.vector.tensor_tensor(dest_ap, t1[:], t2[:], Alu.add)

                qt = [work.tile([DH, CHUNK], BF16, tag="qt", bufs=4,
                                name=f"qt{c}_{h}") for h in range(HPC)]

                def g_q(h):
                    rope(project(wq_sb, h), qt[h][:])

                def g_k(h):
                    kdst = kt_full[h][:, WINDOW + col0:WINDOW + col0 + CHUNK]
                    rope(project(wk_sb, h), kdst)
                    cks = work.tile([DH, CHUNK // RATIO], F32, tag="cks")
                    nc.vector.tensor_reduce(
                        cks[:], kdst.rearrange("p (b r) -> p b r", r=RATIO),
                        AX.X, Alu.add)
                    nc.vector.tensor_scalar_mul(
                        ckt[h][:, col0 // RATIO:(col0 + CHUNK) // RATIO],
                        cks[:], 1.0 / RATIO)

                def g_v(h):
                    acc_v = project(wv_sb, h)
                    vt = work.tile([DH, CHUNK], BF16, tag="vt")
                    nc.scalar.copy(vt[:], acc_v[:])
                    cvs = work.tile([DH, CHUNK // RATIO], F32, tag="cks")
                    nc.vector.tensor_reduce(
                        cvs[:], vt[:].rearrange("p (b r) -> p b r", r=RATIO),
                        AX.X, Alu.add)
                    nc.vector.tensor_scalar_mul(
                        cvt_acc[h][:, col0 // RATIO:(col0 + CHUNK) // RATIO],
                        cvs[:], 1.0 / RATIO)
                    cv_ps = ps.tile([NBLK, DH], BF16, tag="pt", bufs=1)
                    nc.tensor.transpose(cv_ps[:], cvt_acc[h][:], identb[:])
                    nc.vector.tensor_copy(cvrow[h][:], cv_ps[:])
                    vtr_ps = ps.tile([128, CHUNK], BF16, tag="pt", bufs=1)
                    for tt in range(CHUNK // 128):
                        nc.tensor.transpose(vtr_ps[:, 128 * tt:128 * (tt + 1)],
                                            vt[:, 128 * tt:128 * (tt + 1)], identb[:])
                    st0 = CHUNK // 128 * c  # first raw s-tile of this chunk
                    nc.vector.tensor_copy(
                        vrow[h][:, (st0 + 2) * DH:(st0 + 2) * DH + CHUNK], vtr_ps[:])

                groups = [lambda h=h: g_q(h) for h in range(HPC)]
                groups += [lambda h=h: g_k(h) for h in range(HPC)]
                groups += [lambda h=h: g_v(h) for h in range(HPC)]
                return qt, groups

            # ---------- attention block for chunk c (software-pipelined) ----
            # stage A(t): score matmuls + mask-add + exp/accum + recip + diag
            # stage B(t): p-transpose (rhs = diag(1/l)) + pts copy
            # stage C(t): PV matmuls ([dh, q] orientation) + osb copy + DMA
            def att_block(c, qt):
                tiles = []  # (h, tg, qslice)
                for sub in range(CHUNK // 256):
                    sblk = (CHUNK // 256) * c + sub
                    for h in range(HPC):
                        for tt in range(2):
                            tg = 2 * sblk + tt
                            qs = qt[h][:, 256 * sub + 128 * tt:
                                       256 * sub + 128 * (tt + 1)]
                            tiles.append((h, tg, qs))

                state = {}
                fences = {}
                ot_bank = ps.tile([DH, 512], F32, tag="ot", bufs=1,
                                  name=f"otbank{c}")

                def stage_a(i):
                    if ATT < 1:
                        return
                    h, tg, qs = tiles[i]
                    s_ps = ps.tile([128, 512], F32, tag="sc", bufs=2)
                    nc.tensor.matmul(s_ps[:, 0:128], qs, ckt[h][:],
                                     start=True, stop=True)
                    nc.tensor.matmul(s_ps[:, 128:512], qs,
                                     kt_full[h][:, 128 * tg:128 * tg + 384],
                                     start=True, stop=True)
                    if ATT < 2:
                        return
                    # s_sb = scores*scale + mask; logits are bounded (~|8|)
                    # so exp needs no max subtraction.
                    s_sb = work.tile([128, 512], F32, tag="ssb", bufs=2)
                    nc.vector.scalar_tensor_tensor(
                        s_sb[:, 0:128], s_ps[:, 0:128], scale,
                        maskc_sb[:, 120 - 8 * tg:248 - 8 * tg],
                        Alu.mult, Alu.add)
                    nc.vector.scalar_tensor_tensor(
                        s_sb[:, 128:512], s_ps[:, 128:512], scale,
                        maskw_t(tg), Alu.mult, Alu.add)
                    if ATT < 3:
                        return
                    p_sb = work.tile([128, 512], BF16, tag="psb", bufs=2)
                    lsum = work.tile([128, 1], F32, tag="stat", bufs=16)
                    fences["act"] = nc.scalar.activation(
                        p_sb[:], s_sb[:], Act.Exp, bias=0.0, scale=1.0,
                        accum_out=lsum[:])
                    rl = work.tile([128, 1], F32, tag="stat", bufs=16)
                    nc.vector.reciprocal(rl[:], lsum[:])
                    if DBG:
                        nc.sync.dma_start(dbg2_e[tg, h, :, 0:1], lsum[:])
                        nc.sync.dma_start(dbg2_e[tg, h, :, 1:2], rl[:])
                    if ATT < 4:
                        return
                    diag = work.tile([128, 128], BF16, tag="diag", bufs=2)
                    nc.vector.tensor_scalar_mul(diag[:], identb[:], rl[:])
                    state[i] = (p_sb, diag)

                def stage_b(i):
                    if ATT < 5 or i not in state:
                        return
                    p_sb, diag = state[i]
                    if os.environ.get("KERNEL_NODIAG"):
                        diag = identb
                    # pt = p^T @ diag(1/l): transposed AND normalized
                    pt_ps = ps.tile([128, 512], F32, tag="ptb")
                    for j in range(4):
                        nc.tensor.matmul(pt_ps[:, 128 * j:128 * (j + 1)],
                                         p_sb[:, 128 * j:128 * (j + 1)],
                                         diag[:], start=True, stop=True)
                    pts = work.tile([128, 512], BF16, tag="pts", bufs=2)
                    nc.vector.tensor_copy(pts[:], pt_ps[:])
                    state[i] = pts

                def stage_c(i):
                    if ATT < 6 or i not in state:
                        return
                    h, tg, _ = tiles[i]
                    pts = state.pop(i)
                    # PV in [dh, q] orientation: lhsT = v rows ([kv, dh])
                    o_ps = ot_bank[:, 128 * (i % 4):128 * (i % 4 + 1)]
                    nc.tensor.matmul(o_ps, cvrow[h][:], pts[:, 0:128],
                                     start=True, stop=False)
                    for j in range(3):
                        w = tg - 2 + j  # raw s-tile; vrow block w+2
                        nc.tensor.matmul(
                            o_ps, vrow[h][:, (w + 2) * DH:(w + 3) * DH],
                            pts[:, 128 * (j + 1):128 * (j + 2)],
                            start=False, stop=(j == 2))
                    osb = work.tile([DH, 128], BF16, tag="osb", bufs=2,
                                    name="osb")
                    fences["dve"] = nc.vector.tensor_copy(osb[:], o_ps)
                    u, i8 = tg // 8, tg % 8
                    fences["dma"] = nc.sync.dma_start(
                        a2a_in[u][256 * i8 + 128 * h:256 * i8 + 128 * (h + 1), :],
                        osb[:])

                n = len(tiles)

                def make_step(i):
                    def step():
                        if i < n:
                            stage_a(i)
                        if 1 <= i <= n:
                            stage_b(i - 1)
                        if 2 <= i:
                            stage_c(i - 2)
                    return step

                return [make_step(i) for i in range(n + 2)], fences

            def trigger_a2a(u):
                if STAGE < 4:
                    return
                nc.gpsimd.collective_compute(
                    "AllToAll", mybir.AluOpType.bypass,
                    replica_groups=[list(range(NCORES))],
                    ins=[a2a_in[u][:].opt()], outs=[a2a_out[u][:].opt()],
                )

            # ---------- output projection for half u ----------
            def bp_dma(u):
                if STAGE < 3 or STAGE == 4:
                    return
                src_t = a2a_out[u] if STAGE >= 4 else a2a_in[u]
                nc.sync.dma_start(
                    bp_static[u].rearrange("p (k s) -> p k s", k=KT),
                    src_t[:].rearrange("(k p) s -> p k s", p=128))

            def out_proj_groups(u):
                if STAGE < 3 or STAGE == 4:
                    def zero(n):
                        z = work.tile([128, 512], F32, tag="outsb")
                        nc.vector.memset(z[:], 0.0)
                        nc.sync.dma_start(
                            out_e[128 * u:128 * (u + 1),
                                  512 * n:512 * (n + 1)], z[:])
                    return [lambda n=n: zero(n) for n in range(4)]
                bp_sb = bp_static[u]

                def group(n):
                    acc = ps.tile([128, 512], F32, tag="acc")
                    for kk in range(KT):
                        wsb = wo_tiles[2 * n + kk // 8]
                        nc.tensor.matmul(
                            acc[:],
                            bp_sb[:, 128 * kk:128 * (kk + 1)],
                            wsb[:, 512 * (kk % 8):512 * (kk % 8 + 1)],
                            start=(kk == 0), stop=(kk == KT - 1),
                        )
                    outsb = work.tile([128, 512], F32, tag="outsb")
                    nc.vector.tensor_copy(outsb[:], acc[:])
                    nc.sync.dma_start(
                        out_e[128 * u:128 * (u + 1), 512 * n:512 * (n + 1)],
                        outsb[:])
                return [lambda n=n: group(n) for n in range(4)]

            # ---------- schedule ----------
            # interleave attention steps of chunk c with projection groups of
            # chunk c+1: the proj matmuls fill the PE queue while the
            # attention mask/exp chain completes, keeping the ramp hot.
            def interleave(steps, groups, slots=(1, 2, 3, 4, 5, 6)):
                gi = 0
                for i, step in enumerate(steps):
                    step()
                    if i + 1 in slots and gi < len(groups):
                        groups[gi]()
                        gi += 1
                while gi < len(groups):
                    groups[gi]()
                    gi += 1

            qt0, pg0 = proj_block(0)
            for g in pg0:
                g()
            qt1, pg1 = proj_block(1)
            steps0, f0 = att_block(0, qt0)
            interleave(steps0, pg1)
            qt2, pg2 = proj_block(2)
            steps1, f1 = att_block(1, qt1)
            interleave(steps1, pg2)
            trigger_a2a(0)
            qt3, pg3 = proj_block(3)
            steps2, f2 = att_block(2, qt2)
            interleave(steps2, pg3)
            steps3, f3 = att_block(3, qt3)
            for s in steps3:
                s()
            trigger_a2a(1)
            bp_dma(0)
            bp_dma(1)
            for g in out_proj_groups(0):
                g()
            for g in out_proj_groups(1):
                g()
    return nc


def _host_inputs(x, wq, wk, wv, wo):
    """Build per-core input maps (numpy)."""
    import ml_dtypes
    BF = ml_dtypes.bfloat16
    xT = x.reshape(S, D).T.astype(BF)          # [D, S]
    xtile = np.ascontiguousarray(
        xT.reshape(KT, 128, NCHUNK, CHUNK).transpose(2, 1, 0, 3)
        .reshape(NCHUNK, 128, KT * CHUNK))
    woT = wo.T.astype(BF)                      # [D, D]
    wotile = np.ascontiguousarray(
        woT.reshape(2, 8, 128, 4, 512).transpose(3, 0, 2, 1, 4)
        .reshape(8, 128, 8 * 512))

    def wtile(w, rows):
        wT = w[rows, :].T.astype(BF)           # [D, 256]
        return np.ascontiguousarray(
            wT.reshape(KT, 128, 256).transpose(1, 0, 2).reshape(128, KT * 256))

    inv = 1.0 / (ROPE_BASE ** (np.arange(0, DH, 2, dtype=np.float32) / DH))
    theta = np.outer(np.arange(S, dtype=np.float32), inv)  # [S, 64]
    cos = np.cos(theta).T  # [64, S]
    sin = np.sin(theta).T
    COS = np.empty((DH, S), np.float32)
    SINS = np.empty((DH, S), np.float32)
    COS[0::2] = cos
    COS[1::2] = cos
    SINS[0::2] = -sin
    SINS[1::2] = sin

    SWAP = np.zeros((DH, DH), np.float32)
    for t in range(DH // 2):
        SWAP[2 * t + 1, 2 * t] = 1.0
        SWAP[2 * t, 2 * t + 1] = 1.0

    ii = np.arange(128)[:, None]
    cnt = (ii + 1) // RATIO  # [128,1]
    w = np.arange(248)[None, :] - 120
    maskc = np.where(w < cnt, 0.0, NEG).astype(np.float32)  # [128,248]

    col = np.arange(384)[None, :]
    base_vis = (ii < col) & (col <= ii + WINDOW)
    maskw = np.empty((3, 128, 384), np.float32)
    for idx, t in enumerate([0, 1, 2]):
        vis = base_vis & (col >= WINDOW - 128 * t if t < 2 else True)
        maskw[idx] = np.where(vis, 0.0, NEG)

    in_maps = []
    for cid in range(NCORES):
        rows = slice(256 * cid, 256 * (cid + 1))
        in_maps.append({
            "xt": xtile,
            "wqt": wtile(wq, rows),
            "wkt": wtile(wk, rows),
            "wvt": wtile(wv, rows),
            "wot": wotile,
            "cos": COS.astype(BF),
            "sins": SINS.astype(BF),
            "swapm": SWAP.astype(BF),
            "maskc": maskc.astype(BF),
            "maskw": maskw.astype(BF),
        })
    return in_maps


_CACHE = {}
LAST_EXEC_NS = None


def kernel(x, wq, wk, wv, wo):
    _setup_ntff_hook()
    from concourse.bass_utils import run_bass_kernel_spmd

    if "nc" not in _CACHE:
        ncb = build()
        if not ncb.is_finalized():
            ncb.finalize()
        _CACHE["nc"] = ncb
    ncb = _CACHE["nc"]

    in_maps = _host_inputs(np.asarray(x), np.asarray(wq), np.asarray(wk),
                           np.asarray(wv), np.asarray(wo))
    trace = bool(os.environ.get("KERNEL_TRACE"))
    res = run_bass_kernel_spmd(ncb, in_maps, list(range(NCORES)), trace=trace)
    globals()["LAST_EXEC_NS"] = res.exec_time_ns
    # core c owns rows 128c..128c+128 of each s-half
    out = np.empty((S, D), np.float32)
    for c in range(NCORES):
        r = res.results[c]["out"]
        out[128 * c:128 * (c + 1)] = r[0:128]
        out[1024 + 128 * c:1024 + 128 * (c + 1)] = r[128:256]
    return out.reshape(1, S, D)


if __name__ == "__main__":
    rng = np.random.default_rng(0)
    x = rng.standard_normal((1, S, D), dtype=np.float32)
    wq = rng.standard_normal((D, D), dtype=np.float32) * D ** -0.5
    wk = rng.standard_normal((D, D), dtype=np.float32) * D ** -0.5
    wv = rng.standard_normal((D, D), dtype=np.float32) * D ** -0.5
    wo = rng.standard_normal((D, D), dtype=np.float32) * D ** -0.5
    out = kernel(x=x, wq=wq, wk=wk, wv=wv, wo=wo)
    print("out", out.shape, out.dtype, np.abs(out).mean())
